# revision 4
# baseline (speedup 1.0000x reference)
"""Trainium2 Bass kernel for nn_Attention_53850299957994 (B=8, S=2048, D=512).

Data-parallel over batch: one batch element per NeuronCore (8 cores).
The host transposes x and folds the weights into device layouts, runs the
SPMD Bass program via concourse, and stacks the per-core outputs
(fp16 -> fp32).

v3 over the v2 fp16 baseline (190.4us HW / 175.6us sim): single-head
weight folding. scores = (x'Wq^T)(x'Wk^T)^T/sqrt(D) = x'.A.x'^T with
A = Wq^T.Wk/sqrt(D) computed on the host, so the k projection disappears
(x' itself is the key operand of the scores matmul). Likewise
(attn.v).Wd^T = attn.(x'.(Wd.Wv)^T), so the v and d projections fuse into
one. Device matmul work drops from 6 groups (q,k,v,scores,attn,d ~164us
at the fp16 PE rate) to 4 (q~, v'', scores, attn ~137us). Biases fold
exactly: bk's score contribution is constant along the softmax axis
(dropped), bq folds to bq.Wk/sqrt(D) on the q~ projection, bv/bd fold to
Wd.bv + bd post-normalization. Accuracy improves vs v2 (1.04e-3 HW vs
1.10e-3, gate 2e-2) since two quantized projection stages disappear.
fp8 for the big matmuls was measured and rejected: e4m3 operands cost
2.3-3.2e-2 end-to-end (fails the gate), and HW DoubleRow throughput is
~1.7-2x fp16 (the cost model's 4x is wrong), so compensated splits lose.

Schedule (v2 heritage + v3 changes):
- fp16 datapath (pos rides fp8-e4m3), fused just-in-time column DMAs,
  8 warm-up matmuls carrying the PE p-state ramp.
- attention uses exp-chunks as the stationary operand and v'' [seq, feat]
  as moving, so the output accumulates in [query, feat] orientation and
  DMAs straight out - no projection behind it.
- denominators transpose to query-partition orientation via tiny
  ones-matmuls emitted BEFORE the last attention matmuls of each block:
  1/den is ready the moment each output PSUM stops, and the scale is
  fused into staggered Act/DVE PSUM->SBUF copies.
- the last block's four output tiles leave through two fused [P,2,D]
  DMAs on hardware-DGE queues (the single HWDGE issue line serializes at
  ~630ns per DMA, and gpsimd DMAs fall back to slow on-engine SWDGE).
- the first 1024 x/pos columns (2x512-col DMAs) and the q~ weight are
  prefetched: from the
  prelude for a single-shot run, and from the previous body for unrolled
  back-to-back bodies. A body at a For_i back-edge loads inline (a
  prefetch across the loop barrier deadlocks the tile scheduler).
- zero biases (the graded case) skip all bias loads/adds at build time.
"""

from contextlib import ExitStack

import ml_dtypes
import numpy as np

import concourse.bacc as bacc
import concourse.mybir as mybir
import concourse.tile as tile
from concourse.bass_utils import run_bass_kernel_spmd

P = 128
F32 = mybir.dt.float32
F16 = mybir.dt.float16
F8 = mybir.dt.float8e4


def build_nc(S=2048, D=512, IB=512, R=1, with_bias=False, unroll=1, phases=1,
             staggered=False):
    IB = min(IB, S)
    SC = min(512, S)
    DT = D // P            # 4 feature chunks
    ST = S // P            # 16 seq chunks
    NB = S // IB           # 4 i-blocks
    NSC = S // SC          # 4 column stripes
    TPB = IB // P          # 4 i-chunks per i-block
    C0 = 1024              # prefetched first column piece (2 x 512 DMAs)
    Copy = mybir.ActivationFunctionType.Copy
    Ident = mybir.ActivationFunctionType.Identity

    nc = bacc.Bacc("TRN2", target_bir_lowering=False, debug=False, num_devices=8)

    xT = nc.dram_tensor("xT", [D, S], F16, kind="ExternalInput").ap()
    posT = nc.dram_tensor("posT", [D, S], F8, kind="ExternalInput").ap()
    # first pos piece, host-packed dense per partition (two 512-col pieces,
    # each 128 descriptors) so the startup transfer is byte- not desc-bound
    pos0d = nc.dram_tensor("pos0d", [P, 2 * (D // P) * 512], F8,
                           kind="ExternalInput").ap()
    # folded weights: wa = Wq^T.Wk/sqrt(D) [d_in, d], wvd = (Wd.Wv)^T [d_in, e]
    wT = {w: nc.dram_tensor(f"w{w}T", [D, D], F16, kind="ExternalInput").ap()
          for w in ("a", "vd")}
    if with_bias:
        bqs = nc.dram_tensor("bqs", [D], F32, kind="ExternalInput").ap()
        bd = nc.dram_tensor("bd", [D], F32, kind="ExternalInput").ap()
    out = nc.dram_tensor("out", [S, D], F16, kind="ExternalOutput").ap()

    xT_r = xT.rearrange("(o p) s -> p o s", p=P)
    posT_r = posT.rearrange("(o p) s -> p o s", p=P)
    w_r = {w: wT[w].rearrange("(o p) e -> p o e", p=P) for w in ("a", "vd")}
    if with_bias:
        bqs_r = bqs.rearrange("(o p) -> p o", p=P)

    with tile.TileContext(nc) as tc, ExitStack() as ctx:
        persist = ctx.enter_context(tc.tile_pool(name="persist", bufs=1))
        xrpool = ctx.enter_context(tc.tile_pool(name="xrpool", bufs=2))
        pospool = ctx.enter_context(tc.tile_pool(name="pospool", bufs=2))
        expool = ctx.enter_context(tc.tile_pool(name="expool", bufs=3))
        outpool = ctx.enter_context(tc.tile_pool(name="outpool", bufs=4))
        psS = ctx.enter_context(tc.tile_pool(name="psS", bufs=3, space="PSUM"))
        psO = ctx.enter_context(tc.tile_pool(name="psO", bufs=4, space="PSUM"))
        denpool = ctx.enter_context(tc.tile_pool(name="denpool", bufs=1))

        ones_t = None
        ones16_t = None
        pf = {}

        def issue_prefetch(phase):
            # next-iteration startup operands: the first x/pos column piece
            # and the q~ weight. Issued mid-body (or from the prelude for
            # iteration 0) so the body's first adds/matmuls never wait on
            # the HWDGE issue line after the For_i barrier.
            sfx = f"_p{phase}" if phases > 1 else ""
            x0 = persist.tile([P, DT, C0], F16, tag=f"x0pf{sfx}",
                              name=f"x0pf{sfx}")
            pos0 = persist.tile([P, 2, DT, C0 // 2], F8, tag=f"pos0pf{sfx}",
                                name=f"pos0pf{sfx}")
            wa = persist.tile([P, DT, D], F16, tag=f"wa{sfx}",
                              name=f"wa{sfx}")
            pos0d_r = pos0d.rearrange("p (t x) -> p t x", t=2)
            h = C0 // 2
            # first-512 pieces and wa half A gate the body's first matmuls;
            # the second pieces ride behind them on the same queues
            nc.sync.dma_start(out=x0[:, :, 0:h], in_=xT_r[:, :, 0:h])
            nc.scalar.dma_start(out=pos0[:, 0].rearrange("p o s -> p (o s)"),
                                in_=pos0d_r[:, 0])
            nc.sync.dma_start(out=wa[:, 0:2, :], in_=w_r["a"][:, 0:2, :])
            nc.scalar.dma_start(out=wa[:, 2:4, :], in_=w_r["a"][:, 2:4, :])
            nc.sync.dma_start(out=x0[:, :, h:C0], in_=xT_r[:, :, h:C0])
            nc.scalar.dma_start(out=pos0[:, 1].rearrange("p o s -> p (o s)"),
                                in_=pos0d_r[:, 1])
            pf[phase] = (x0, pos0, wa)

        def prelude():
            # runs once, before the (optional) repeat loop
            nonlocal ones_t, ones16_t
            ones_t = persist.tile([P, 1], F32, tag="ones", name="ones")
            nc.gpsimd.memset(ones_t, 1.0)
            # DVE is idle at t=0; gpsimd's preamble memsets would delay this
            # by ~0.7us and with it the warm-up start
            ones16_t = persist.tile([P, 1], F16, tag="ones16", name="ones16")
            nc.vector.memset(ones16_t, 1.0)
            if R == 1:
                issue_prefetch(0)
            # p-state warm-up: broadcast-operand matmuls keep the PE busy
            # from the preamble until the first real operands land, so the
            # first projections run at full clock instead of ramping
            warm_ps = psS.tile([P, SC], F32, tag="S", name="warm")
            wsrc = ones16_t.to_broadcast((P, P))
            wmov = ones16_t.to_broadcast((P, SC))
            for _ in range(8):
                nc.tensor.matmul(warm_ps, wsrc, wmov, start=True, stop=True)

        def body(phase=0, use_pf=True, reissue=True):
            wt = {}
            sfx = f"_p{phase}" if phases > 1 else ""

            def load_w(which, engA, engB=None):
                t = persist.tile([P, DT, D], F16, tag=f"w{which}{sfx}",
                                 name=f"w{which}{sfx}")
                engB = engB or engA
                engA.dma_start(out=t[:, 0:2, :], in_=w_r[which][:, 0:2, :])
                engB.dma_start(out=t[:, 2:4, :], in_=w_r[which][:, 2:4, :])
                wt[which] = t

            # --- startup: when this body follows another in straight-line
            # code, the first x/pos piece and the q~ weight were prefetched
            # and the first adds/matmuls start right away. A body at a
            # For_i back-edge must load inline (a prefetch crossing the
            # loop barrier deadlocks the scheduler).
            xp = persist.tile([P, DT, S], F16, tag=f"xp{sfx}", name=f"xp{sfx}")
            if not use_pf:
                issue_prefetch(phase)
            x0, pos0, wa_t = pf[phase]
            wt["a"] = wa_t
            # per-(piece, o) adds: the first projection matmul only needs the
            # first 512-col o=0 chunk of xp, so it starts as early as possible
            h = C0 // 2
            for t in range(2):
                for o in range(DT):
                    nc.vector.tensor_add(
                        out=xp[:, o, t * h:(t + 1) * h],
                        in0=x0[:, o, t * h:(t + 1) * h],
                        in1=pos0[:, t, o, :])
            if with_bias:
                bqs_t = persist.tile([P, DT], F32, tag=f"bqs{sfx}")
                nc.gpsimd.dma_start(out=bqs_t, in_=bqs_r)
                bd_bc = persist.tile([P, D], F32, tag=f"bd_bc{sfx}")
                nc.gpsimd.dma_start(out=bd_bc,
                                    in_=bd.unsqueeze(0).to_broadcast((P, D)))

            # streamed x/pos columns: fused [P, DT, w] transfers + DVE adds,
            # issued just-in-time (one column ahead) so no DGE ring ever
            # backs up and blocks a sequencer mid-kernel.
            loaded = set()

            def load_col(lo, w):
                if lo in loaded:
                    return
                loaded.add(lo)
                xr = xrpool.tile([P, DT, w], F16, tag="xr", name=f"x{lo}")
                nc.sync.dma_start(out=xr, in_=xT_r[:, :, lo:lo + w])
                pr = pospool.tile([P, DT, w], F8, tag="pos", name=f"p{lo}")
                nc.scalar.dma_start(out=pr, in_=posT_r[:, :, lo:lo + w])
                for o in range(DT):
                    nc.vector.tensor_add(out=xp[:, o, lo:lo + w],
                                         in0=xr[:, o, :], in1=pr[:, o, :])

            # --- q~ projection, chunked columns ---
            qt = {}
            for et in range(DT):
                for sc in range(NSC):
                    qt[(et, sc)] = persist.tile([P, SC], F16,
                                                tag=f"q{et}_{sc}{sfx}",
                                                name=f"q{et}_{sc}{sfx}")

            def proj_chunk(dst, sc, lo, w, b_t):
                for et in range(DT):
                    ps = psS.tile([P, SC], F32, tag="S")
                    r = ps[:, lo - sc * SC:lo - sc * SC + w]
                    for o in range(DT):
                        nc.tensor.matmul(
                            r, wt["a"][:, o, et * P:(et + 1) * P],
                            xp[:, o, lo:lo + w],
                            start=(o == 0), stop=(o == DT - 1))
                    dr = dst[(et, sc)][:, lo - sc * SC:lo - sc * SC + w]
                    if with_bias:
                        nc.scalar.activation(out=dr, in_=r, func=Ident,
                                             bias=b_t[:, et:et + 1], scale=1.0)
                    else:
                        nc.scalar.activation(out=dr, in_=r, func=Copy)

            chunks = [(sc, sc * SC, SC) for sc in range(NSC)]
            stream = [(lo, SC) for lo in range(C0, S, SC)]
            for ci, (sc, lo, w) in enumerate(chunks):
                if ci < len(stream):
                    load_col(*stream[ci])
                if ci == 1:
                    load_w("vd", nc.sync, nc.scalar)
                proj_chunk(qt, sc, lo, w, bqs_t if with_bias else None)
            # issue the startup prefetch for the next straight-line body
            # (the DMA writes wait on this body's last x0/wa reads via the
            # pool WAR semaphores; transfers land long before the body ends)
            if reissue:
                issue_prefetch((phase + 1) % phases)

            # --- v'' projection (seq chunks onto partitions) ---
            vt = {}
            sps_q = {}
            emitted = set()

            def emit_scores(ib, jt):
                # scores tile [128 j, IB i] = sum_o xp_chunk^T . q~
                sps = psS.tile([P, IB], F32, tag="S", name=f"sps{ib}_{jt}")
                for o in range(DT):
                    nc.tensor.matmul(
                        sps,
                        xp[:, o, jt * P:(jt + 1) * P],
                        qt[(o, ib)],
                        start=(o == 0),
                        stop=(o == DT - 1),
                    )
                sps_q[(ib, jt)] = sps

            def emit_next(ib, jt):
                if (ib, jt) not in emitted and ib < NB:
                    emitted.add((ib, jt))
                    emit_scores(ib, jt)

            for jt in range(ST):
                if jt == ST - 2:
                    emit_next(0, 0)  # warm the scores pipe under the vproj tail
                ps = psO.tile([P, D], F32, tag="O")
                for o in range(DT):
                    nc.tensor.matmul(
                        ps,
                        xp[:, o, jt * P:(jt + 1) * P],
                        wt["vd"][:, o, :],
                        start=(o == 0),
                        stop=(o == DT - 1),
                    )
                vj = persist.tile([P, D], F16, tag=f"v{jt}{sfx}",
                                  name=f"v{jt}{sfx}")
                nc.scalar.activation(out=vj, in_=ps, func=Copy)
                vt[jt] = vj

            # --- attention + PE denominator transpose, direct output ---
            emit_next(0, 0)
            mult = mybir.AluOpType.mult
            add = mybir.AluOpType.add
            for ib in range(NB):
                yps = [psO.tile([P, D], F32, tag="O", name=f"yps{ic}")
                       for ic in range(TPB)]
                acc = denpool.tile([P, IB], F32, tag=f"acc{sfx}")
                den_ps = None
                for jt in range(ST):
                    if jt + 1 < ST:
                        emit_next(ib, jt + 1)
                    elif ib + 1 < NB:
                        emit_next(ib + 1, 0)  # keep PE fed across the block edge
                    ex = expool.tile([P, IB], F16, tag="exp")
                    nc.scalar.activation(
                        out=ex, in_=sps_q.pop((ib, jt)),
                        func=mybir.ActivationFunctionType.Exp,
                    )
                    if jt == ST - 1:
                        # denominators to query-partition orientation via tiny
                        # matmuls, emitted BEFORE the last attn matmuls: rT is
                        # then ready the moment each yps[ii] stops, so the
                        # output copies start staggered right behind the PE.
                        # (the last j-tile's exp feeds in directly, off the
                        # DVE acc chain)
                        den_ps = psS.tile([P, TPB], F32, tag="S",
                                          name=f"den{ib}")
                        for ii in range(TPB):
                            nc.tensor.matmul(
                                den_ps[:, ii:ii + 1],
                                acc[:, ii * P:(ii + 1) * P],
                                ones_t,
                                start=True, stop=False)
                            nc.tensor.matmul(
                                den_ps[:, ii:ii + 1],
                                ex[:, ii * P:(ii + 1) * P],
                                ones16_t,
                                start=False, stop=True)
                    for ic in range(TPB):
                        nc.tensor.matmul(
                            yps[ic],
                            ex[:, ic * P:(ic + 1) * P],
                            vt[jt],
                            start=(jt == 0),
                            stop=(jt == ST - 1),
                        )
                    if jt == 0:
                        nc.vector.tensor_copy(out=acc, in_=ex)
                    elif jt < ST - 1:
                        nc.vector.tensor_add(out=acc, in0=acc, in1=ex)

                rT = denpool.tile([P, TPB], F32, tag=f"rT{sfx}")
                nc.vector.reciprocal(out=rT, in_=den_ps)

                if ib + 1 < NB:
                    emit_next(ib + 1, 1)  # PE work before the output block
                last_b = ib == NB - 1
                if last_b and not with_bias:
                    # tail: staggered Act/DVE copies into one [P, TPB, D]
                    # tile, drained by two fused DMAs (halves the HWDGE
                    # issue serialization at the very end of the kernel)
                    f4 = outpool.tile([P, TPB, D], F16, tag="fout4")
                    for ii in range(TPB):
                        if ii % 2 == 0:
                            nc.scalar.activation(out=f4[:, ii, :],
                                                 in_=yps[ii], func=Copy,
                                                 scale=rT[:, ii:ii + 1])
                        else:
                            nc.vector.tensor_scalar_mul(
                                out=f4[:, ii, :], in0=yps[ii],
                                scalar1=rT[:, ii:ii + 1])
                        if ii == 1:
                            nc.sync.dma_start(
                                out=out[ib * IB:ib * IB + 2 * P, :].rearrange(
                                    "(o p) e -> p o e", p=P),
                                in_=f4[:, 0:2, :])
                        elif ii == 3:
                            # scalar queue (HWDGE): gpsimd would fall back to
                            # the slow SWDGE on-engine descriptor path
                            nc.scalar.dma_start(
                                out=out[ib * IB + 2 * P:(ib + 1) * IB, :]
                                .rearrange("(o p) e -> p o e", p=P),
                                in_=f4[:, 2:4, :])
                else:
                    for ii in range(TPB):
                        it = ib * TPB + ii
                        f_sb = outpool.tile([P, D], F16, tag="fout")
                        if with_bias:
                            nc.vector.scalar_tensor_tensor(
                                out=f_sb, in0=yps[ii], scalar=rT[:, ii:ii + 1],
                                in1=bd_bc, op0=mult, op1=add)
                        elif ii % 2 == 0:
                            nc.scalar.activation(out=f_sb, in_=yps[ii],
                                                 func=Copy,
                                                 scale=rT[:, ii:ii + 1])
                        else:
                            nc.vector.tensor_scalar_mul(out=f_sb, in0=yps[ii],
                                                        scalar1=rT[:, ii:ii + 1])
                        qmap = (nc.scalar, nc.sync, nc.gpsimd, nc.sync)
                        qmap[ii].dma_start(
                            out=out[it * P:(it + 1) * P, :], in_=f_sb)
                if ib + 1 < NB:
                    emit_next(ib + 1, 2)  # more PE runahead over the edge
                    emit_next(ib + 1, 3)

        prelude()

        def all_bodies():
            for u in range(unroll):
                body(u % phases,
                     use_pf=(u > 0 or R == 1),
                     reissue=(u < unroll - 1))

        if R == 1:
            all_bodies()
        else:
            with tc.For_i(0, R, 1, staggered_reset=staggered,
                          hint_engines=(
                              mybir.EngineType.PE,
                              mybir.EngineType.Activation,
                              mybir.EngineType.DVE)):
                all_bodies()

    nc.compile()
    return nc


def host_prep(x, pos_table, Wq, bq, Wk, bk, Wv, bv, Wd, bd):
    B, S, D = x.shape
    f = np.float32
    h = np.float16
    with_bias = bool(np.any(np.asarray(bq)) or np.any(np.asarray(bk))
                     or np.any(np.asarray(bv)) or np.any(np.asarray(bd)))
    pt8 = np.ascontiguousarray(
        np.asarray(pos_table, dtype=f)[:S].T).astype(ml_dtypes.float8_e4m3)

    def pack0(a):  # [D, w] -> [128, (D//128)*w], partition-major dense
        w = a.shape[1]
        return (a.reshape(D // 128, 128, w).transpose(1, 0, 2)
                .reshape(128, -1))
    Wqf = np.asarray(Wq, dtype=f)
    Wkf = np.asarray(Wk, dtype=f)
    Wvf = np.asarray(Wv, dtype=f)
    Wdf = np.asarray(Wd, dtype=f)
    A = (Wqf.T @ Wkf) / np.sqrt(np.float32(D))       # [d_in, d]
    Wvd = np.ascontiguousarray((Wdf @ Wvf).T)         # [d_in, e]
    shared = {
        "posT": pt8,
        "pos0d": np.ascontiguousarray(np.concatenate(
            [pack0(pt8[:, 0:512]), pack0(pt8[:, 512:1024])], axis=1)),
        "waT": np.ascontiguousarray(A).astype(h),
        "wvdT": Wvd.astype(h),
    }
    if with_bias:
        # bk drops out entirely: its score contribution is constant along
        # the softmax axis. bq folds onto the q~ projection; bv/bd fold to
        # a post-normalization add.
        shared["bqs"] = (np.asarray(bq, dtype=f) @ Wkf) / np.sqrt(np.float32(D))
        shared["bd"] = (np.asarray(bd, dtype=f) + Wdf @ np.asarray(bv, dtype=f))
    in_maps = []
    for b in range(B):
        m = dict(shared)
        m["xT"] = np.ascontiguousarray(np.asarray(x[b], dtype=f).T).astype(h)
        in_maps.append(m)
    return in_maps, with_bias


_NC_CACHE = {}


def _get_nc(S, D, R=1, with_bias=False):
    key = (S, D, R, with_bias)
    if key not in _NC_CACHE:
        _NC_CACHE[key] = build_nc(S=S, D=D, R=R, with_bias=with_bias)
    return _NC_CACHE[key]


def kernel(x, pos_table, Wq, bq, Wk, bk, Wv, bv, Wd, bd):
    """Full inputs -> full output [B, S, D], computed on 8 NeuronCores."""
    x = np.asarray(x)
    B, S, D = x.shape
    assert B == 8, f"expected B=8, got {B}"
    in_maps, with_bias = host_prep(x, np.asarray(pos_table), np.asarray(Wq),
                                   np.asarray(bq), np.asarray(Wk),
                                   np.asarray(bk), np.asarray(Wv),
                                   np.asarray(bv), np.asarray(Wd),
                                   np.asarray(bd))
    nc = _get_nc(S, D, with_bias=with_bias)
    res = run_bass_kernel_spmd(nc, in_maps, core_ids=list(range(B)))
    return np.stack([res.results[b]["out"] for b in range(B)]).astype(np.float32)
